# revision 22
# baseline (speedup 1.0000x reference)
"""BiAttentionLayer Trainium2 kernel (Bass/Tile), data-parallel over batch N.

Full inputs:  H [64,1024,200], U [64,64,200], c_mask [64,1024],
              q_mask [64,64], w [600], b []
Full output:  G [64,1024,800] = concat([H, U_, H*U_, H*H_], -1)

Sharding: batch rows 8 per core across 8 NeuronCores; masks/w/b replicated.

Math (matches the reference to bf16 rounding; gate is rel_err < 2e-2):
  S = (H@w_h)[:,:,None] + (U@w_u)[:,None,:] + (H*w_hu)@U^T + b
  masked_softmax(v,m) == exp(v*m)*m / sum_j(...); the C2Q normalization is
  invariant to any per-t factor, so with NEG=100:
    e[t,j] = exp((S[t,j]+100)*qm[j] - 100)  (masked lanes underflow to ~0)
    U_ = (e @ U) / sum_j e,   rt = max_j(e)*cm,  a = rt/sum_t rt, H_ = a@H.

S is computed TRANSPOSED: S'[j,t] = sum_d uwq[d,j] * H^T[d,t] with
  uwq[d,j] = (w_hu[d]*U[j,d] + w_h[d]) * qm[j]          (folds S1, S3, mask)
  bias[j]  = (S2[j] + b + 100) * qm[j] - 100            (ACT exp bias column)
so e'[j,t] = exp(S' + bias) is ONE activation op and e' is directly the lhsT
of the U_ matmul; a small PE transpose recovers [t,j] just for the row max.
All PE operands are bf16; PSUM accumulation and H/U_/G stay fp32.

Perf notes driving this shape (HW-measured): every DVE/ACT instruction costs
~350-600ns of overhead regardless of width, and the PE runs at the 1.2GHz
throttled clock, so the kernel minimizes INSTRUCTION COUNT above all:
 - chunks are processed in QUADS (512 t-rows per instruction where possible:
   one Hb cast, one ht drain, one exp, one reduce_max, one H*U_ per quad)
 - all per-row setup (U^T, uwq weights, S2/bias columns, masks) is batched
   into a handful of whole-problem instructions at kernel start
 - the contraction splits d as 0:128 / 72:200 so every transpose and hbar
   matmul is a full 128-col weight load (FWL-eligible); the overlapping
   d-range 72:128 of block 2 is zeroed in the uwq weights via a mask column
 - DMA: one 819KB H load per row (ACT HWDGE ring) and two 1.6MB half-row G
   stores (SP ring) so loads overlap stores; 4 row buffers make the
   write-after-read wait on a reused buffer ~0.
"""

import os
import sys

for _p in ("/opt/trn_rl_repo", "/root/.axon_site/_ro/trn_rl_repo"):
    if os.path.isdir(_p) and _p not in sys.path:
        sys.path.insert(0, _p)

import numpy as np

import concourse.bass as bass
import concourse.tile as tile
from concourse import mybir
from concourse.masks import make_identity

N_CORES = 8
N_FULL = 64
B = N_CORES and N_FULL // N_CORES   # 8 batch rows per core
T = 1024
J = 64
D2 = 200
DG = 4 * D2                    # 800
NCHUNK = T // 128              # 8
NEG_SOFT = 100.0               # exp(x - 100): masked lanes underflow to ~0
KO = 72                        # block-2 d-offset: block1 = d 0:128, block2 = d 72:200

FP = mybir.dt.float32
BF = mybir.dt.bfloat16
MULT = mybir.AluOpType.mult
ADD = mybir.AluOpType.add
AXX = mybir.AxisListType.X
EXP = mybir.ActivationFunctionType.Exp
COPYF = mybir.ActivationFunctionType.Copy


def _split_overwide_waits(nc, max_waits=1):
    """This walrus build only encodes one semaphore wait per instruction;
    hoist extra waits onto no-ops just before the offending instruction."""
    for bb in nc.m.functions[0].blocks:
        i = 0
        while i < len(bb.instructions):
            ins = bb.instructions[i]
            si = getattr(ins, "sync_info", None)
            if si is not None and si.on_wait is not None and len(si.on_wait) > max_waits:
                waits = list(si.on_wait)
                si.on_wait = waits[-max_waits:]
                rest = waits[:-max_waits]
                k = 0
                while rest:
                    chunk, rest = rest[:max_waits], rest[max_waits:]
                    nop = mybir.InstNoOp(
                        name=f"{ins.name}-wsplit{k}",
                        engine=ins.engine,
                        bass_nofuse=True,
                        sync_info=mybir.SyncInfo(on_wait=chunk, on_update=[]),
                    )
                    bb.instructions.insert(i, nop)
                    i += 1
                    k += 1
            i += 1


def build_program(split_waits=True):
    nc = bass.Bass()

    H_d = nc.dram_tensor("H", [B, T, D2], FP, kind="ExternalInput")
    U_d = nc.dram_tensor("U", [B, J, D2], FP, kind="ExternalInput")
    cm_d = nc.dram_tensor("c_mask", [B, T], FP, kind="ExternalInput")
    qm_d = nc.dram_tensor("q_mask", [B, J], FP, kind="ExternalInput")
    w_d = nc.dram_tensor("w", [3 * D2], FP, kind="ExternalInput")
    b_d = nc.dram_tensor("b", [1, 1], FP, kind="ExternalInput")
    G_d = nc.dram_tensor("G", [B, T, DG], FP, kind="ExternalOutput")

    with tile.TileContext(nc) as tc:
        with (
            tc.tile_pool(name="const", bufs=1) as constp,
            tc.tile_pool(name="row", bufs=2) as rowp,
            tc.tile_pool(name="grow", bufs=4) as growp,
            tc.tile_pool(name="hb", bufs=4) as hbp,
            tc.tile_pool(name="chunk", bufs=3) as chp,
            tc.tile_pool(name="ps_tr", bufs=2, space="PSUM") as ptrp,
            tc.tile_pool(name="ps_s", bufs=2, space="PSUM") as ps_sp,
            tc.tile_pool(name="ps_u", bufs=1, space="PSUM") as ps_up,
            tc.tile_pool(name="ps_e", bufs=1, space="PSUM") as ps_ep,
            tc.tile_pool(name="ps_row", bufs=1, space="PSUM") as rowps,
        ):
            # ================= constants & whole-problem setup =================
            identf = constp.tile([128, 128], FP)
            make_identity(nc, identf)
            identb = constp.tile([128, 128], BF)
            nc.vector.tensor_copy(out=identb, in_=identf)
            ones_row = constp.tile([1, 128], BF)
            nc.vector.memset(ones_row, 1.0)
            # zmask zeroes the duplicated d-range 72:128 in block-2 weights
            zmask = constp.tile([128, 1], FP)
            nc.vector.memset(zmask, 1.0)
            nc.vector.memset(zmask[0:128 - KO, 0:1], 0.0)

            b64 = constp.tile([J, 1], FP)
            nc.gpsimd.dma_start(out=b64, in_=b_d[:, :].partition_broadcast(J))
            b100 = constp.tile([J, 1], FP)
            nc.vector.tensor_scalar_add(out=b100, in0=b64, scalar1=NEG_SOFT)

            # w columns on the overlapped split: block1 = d 0:128, block2 = d 72:200
            wh1 = constp.tile([128, 1], FP)
            wh2 = constp.tile([128, 1], FP)
            whu1 = constp.tile([128, 1], FP)
            whu2 = constp.tile([128, 1], FP)
            wu1b = constp.tile([128, 1], BF)
            wu2b = constp.tile([128, 1], BF)
            for sb, lo in ((wh1, 0), (wh2, KO), (whu1, 2 * D2), (whu2, 2 * D2 + KO),
                           (wu1b, D2), (wu2b, D2 + KO)):
                nc.gpsimd.dma_start(out=sb, in_=w_d[lo:lo + 128].unsqueeze(1))
            # zero the duplicated d-range in the block-2 wu column (S2 matmul)
            nc.vector.tensor_scalar_mul(out=wu2b, in0=wu2b, scalar1=zmask[:, 0:1])

            qm_b = constp.tile([128, B * J], BF)     # [p, r*64+j] = qm[r, j]
            nc.gpsimd.dma_start(
                out=qm_b, in_=qm_d.rearrange("r j -> (r j)").partition_broadcast(128)
            )
            qm_col = constp.tile([J, B], FP)         # [j, r]
            nc.gpsimd.dma_start(out=qm_col, in_=qm_d.rearrange("r j -> j r"))
            U_all = constp.tile([J, B * D2], FP)     # [j, r*200+d]
            nc.sync.dma_start(
                out=U_all.rearrange("j (r d) -> j r d", d=D2),
                in_=U_d.rearrange("r j d -> j r d"),
            )

            # t-rows are mapped t = 8p + c (chunk index c INNER) so that a
            # 4-chunk quad of G is 12.8KB contiguous DRAM per partition.
            # cmT[p, r*8+c] = c_mask[r, 8p+c], loaded directly in that layout.
            cmTf = constp.tile([128, B * NCHUNK], FP)
            nc.gpsimd.dma_start(
                out=cmTf.rearrange("p (r c) -> p r c", c=NCHUNK),
                in_=cm_d.rearrange("r (p c) -> p r c", c=NCHUNK),
            )
            cmT = constp.tile([128, B * NCHUNK], BF)
            nc.vector.tensor_copy(out=cmT, in_=cmTf)

            # Ub_all: bf16 copy of U with a ones column per row (denominator)
            UB1 = D2 + 1
            Ub_all = constp.tile([J, B * UB1], BF)   # [j, r*201 + d], col 200 = 1
            Ub_v = Ub_all.rearrange("j (r u) -> j r u", u=UB1)
            nc.scalar.copy(
                out=Ub_v[:, :, 0:D2],
                in_=U_all.rearrange("j (r d) -> j r d", d=D2),
            )
            nc.vector.memset(Ub_v[:, :, D2:UB1], 1.0)

            # UT_all[d, r*128 + (blk*64 + j)] = U[r, j, dblk]  (16 transposes)
            UT_all = constp.tile([128, B * 128], BF)
            for half in range(2):
                utps = rowps.tile([128, 4 * 128], BF, tag="row")
                for i in range(4):
                    r = half * 4 + i
                    nc.tensor.transpose(
                        utps[:, i * 128:i * 128 + J],
                        Ub_all[:, r * UB1:r * UB1 + 128], identb[0:J, 0:J]
                    )
                    nc.tensor.transpose(
                        utps[:, i * 128 + J:(i + 1) * 128],
                        Ub_all[:, r * UB1 + KO:r * UB1 + D2], identb[0:J, 0:J]
                    )
                nc.vector.tensor_copy(
                    out=UT_all[:, half * 512:(half + 1) * 512], in_=utps
                )

            # uwq_all[d, r*128 + blk*64 + j] = (whu[d]*U^T + wh[d]) * qm[j]
            # (block 2 additionally zeroed on the duplicated d-range via zmask)
            uwq_all = constp.tile([128, B * 128], BF)
            uw3 = uwq_all.rearrange("d (r x) -> d r x", x=128)
            ut3 = UT_all.rearrange("d (r x) -> d r x", x=128)
            nc.vector.tensor_scalar(
                out=uw3[:, :, 0:J], in0=ut3[:, :, 0:J],
                scalar1=whu1[:, 0:1], scalar2=wh1[:, 0:1], op0=MULT, op1=ADD,
            )
            nc.vector.tensor_scalar(
                out=uw3[:, :, J:2 * J], in0=ut3[:, :, J:2 * J],
                scalar1=whu2[:, 0:1], scalar2=wh2[:, 0:1], op0=MULT, op1=ADD,
            )
            qm_bv = qm_b.rearrange("d (r j) -> d r j", j=J)
            nc.vector.tensor_tensor(
                out=uw3[:, :, 0:J], in0=uw3[:, :, 0:J],
                in1=qm_bv, op=MULT,
            )
            nc.vector.scalar_tensor_tensor(
                out=uw3[:, :, J:2 * J], in0=uw3[:, :, J:2 * J],
                scalar=zmask[:, 0:1], in1=qm_bv, op0=MULT, op1=MULT,
            )

            # S2_all[j, r] = U[r] @ w_u, then bias_all = (S2+b+100)*qm - 100
            S2ps = rowps.tile([J, B], FP, tag="row")
            for r in range(B):
                nc.tensor.matmul(
                    S2ps[:, r:r + 1], UT_all[:, r * 128:r * 128 + J], wu1b,
                    start=True, stop=False,
                )
                nc.tensor.matmul(
                    S2ps[:, r:r + 1], UT_all[:, r * 128 + J:(r + 1) * 128], wu2b,
                    start=False, stop=True,
                )
            bias_all = constp.tile([J, B], FP)
            nc.vector.scalar_tensor_tensor(
                out=bias_all, in0=S2ps, scalar=b100[:, 0:1],
                in1=qm_col, op0=ADD, op1=MULT,
            )
            nc.vector.tensor_scalar_add(
                out=bias_all, in0=bias_all, scalar1=-NEG_SOFT
            )

            # ================= per-row / per-quad stages =================

            def load_grow(r):
                # H loads ride the ACT HWDGE ring so they overlap stores
                g = growp.tile([128, NCHUNK * DG], FP, tag="g")
                gv = g.rearrange("p (c gg) -> p c gg", gg=DG)
                nc.scalar.dma_start(
                    out=gv[:, :, 0:D2],
                    in_=H_d[r].rearrange("(p c) d -> p c d", c=NCHUNK),
                )
                return g

            def store_full(r, g, q):
                # quad slice is contiguous on both sides: 12.8KB descriptors
                gd = G_d[r].rearrange("(p c) gg -> p c gg", c=NCHUNK)
                gs = g.rearrange("p (c gg) -> p c gg", gg=DG)
                nc.sync.dma_start(
                    out=gd[:, 4 * q:4 * q + 4, :], in_=gs[:, 4 * q:4 * q + 4, :]
                )

            def headA(st, q):
                g = st["g"]
                gv = g.rearrange("p (c gg) -> p c gg", gg=DG)
                Hb = hbp.tile([128, 4 * D2], BF, tag="hb")
                st["Hb"][q] = Hb
                nc.scalar.copy(
                    out=Hb.rearrange("p (k d) -> p k d", d=D2),
                    in_=gv[:, 4 * q:4 * q + 4, 0:D2],
                )

            def headB(st, q):
                r = st["r"]
                Hb = st["Hb"][q]
                # trc cols: [k*128 : k*128+128] = block1 of chunk k (d 0:128),
                #           [512 + k*128 : ...] = block2 (d 72:200)
                trc = ptrp.tile([128, 1024], BF, tag="tr")
                for k in range(4):
                    nc.tensor.transpose(
                        trc[:, k * 128:(k + 1) * 128],
                        Hb[:, k * D2:k * D2 + 128], identb,
                    )
                for k in range(4):
                    nc.tensor.transpose(
                        trc[:, 512 + k * 128:512 + (k + 1) * 128],
                        Hb[:, k * D2 + KO:(k + 1) * D2], identb,
                    )
                ht = chp.tile([128, 1024], BF, tag="ht")
                nc.vector.tensor_copy(out=ht, in_=trc)
                ps_s = ps_sp.tile([J, 512], FP, tag="s")
                st["ps_s"][q] = ps_s
                nc.tensor.matmul(
                    ps_s, uwq_all[:, r * 128:r * 128 + J], ht[:, 0:512],
                    start=True, stop=False,
                )
                nc.tensor.matmul(
                    ps_s, uwq_all[:, r * 128 + J:(r + 1) * 128], ht[:, 512:1024],
                    start=False, stop=True,
                )

            def soft(st, q):
                r = st["r"]
                eT = chp.tile([J, 512], BF, tag="eT")
                st["eT"][q] = eT
                nc.scalar.activation(
                    out=eT, in_=st["ps_s"][q], func=EXP,
                    bias=bias_all[:, r:r + 1], scale=1.0,
                )

            def tail(st, q):
                r = st["r"]
                g = st["g"]
                gv = g.rearrange("p (c gg) -> p c gg", gg=DG)
                eT = st["eT"][q]
                Ub_r = Ub_all[:, r * UB1:(r + 1) * UB1]
                eP = ps_ep.tile([128, 4 * J], BF, tag="e")
                # one 2-bank psU tile, chunk regions at a uniform 256-f32
                # stride (201 used + pad) so none crosses a bank boundary and
                # reciprocal + U_ scale each run as ONE op per quad
                psU = ps_up.tile([128, 4 * 256], FP, tag="u")
                for k in range(4):
                    nc.tensor.matmul(
                        psU[:, k * 256:k * 256 + UB1],
                        eT[:, k * 128:(k + 1) * 128], Ub_r,
                        start=True, stop=True,
                    )
                for k in range(4):
                    nc.tensor.transpose(
                        eP[:, k * J:(k + 1) * J],
                        eT[:, k * 128:(k + 1) * 128], identb[0:J, 0:J],
                    )
                psUv = psU.rearrange("p (k u) -> p k u", u=256)
                rp = chp.tile([128, 4], FP, tag="rp")
                nc.vector.reciprocal(
                    out=rp.rearrange("p (k o) -> p k o", o=1),
                    in_=psUv[:, :, D2:UB1],
                )
                nc.vector.tensor_tensor(
                    out=gv[:, 4 * q:4 * q + 4, D2:2 * D2],
                    in0=psUv[:, :, 0:D2],
                    in1=rp.rearrange("p (k o) -> p k o", o=1).broadcast_to(
                        [128, 4, D2]),
                    op=MULT,
                )
                nc.vector.reduce_max(
                    st["rt_raw"][:, 4 * q:4 * q + 4],
                    eP.rearrange("p (k j) -> p k j", j=J), axis=AXX,
                )
                nc.vector.tensor_tensor(
                    out=gv[:, 4 * q:4 * q + 4, 2 * D2:3 * D2],
                    in0=gv[:, 4 * q:4 * q + 4, 0:D2],
                    in1=gv[:, 4 * q:4 * q + 4, D2:2 * D2], op=MULT,
                )

            def rowend(st):
                r = st["r"]
                rt = st["rt"]
                nc.vector.tensor_tensor(
                    out=rt.rearrange("p (c o) -> p c o", o=1),
                    in0=st["rt_raw"].rearrange("p (c o) -> p c o", o=1),
                    in1=cmT.rearrange("p (rr c) -> p c rr", c=NCHUNK)[:, :, r:r + 1],
                    op=MULT,
                )
                # hbar^T columns: col0 = d 0:128, col1 = d 72:200 (rows 56:128
                # hold d 128:200; rows 0:56 are computed but unused)
                ps_h = rowps.tile([128, 2], FP, tag="row")
                for blk in range(2):
                    off = 0 if blk == 0 else KO
                    for q in range(2):
                        for k in range(4):
                            c = 4 * q + k
                            nc.tensor.matmul(
                                ps_h[:, blk:blk + 1],
                                st["Hb"][q][:, k * D2 + off:k * D2 + off + 128],
                                rt[:, c:c + 1],
                                start=(c == 0), stop=(c == NCHUNK - 1),
                            )
                rtp = rowp.tile([128, 1], FP, tag="rtp")
                nc.vector.reduce_sum(rtp, rt, axis=AXX)
                hbc = rowp.tile([128, 2], BF, tag="hbc")
                nc.scalar.copy(out=hbc, in_=ps_h)
                rtpT = rowps.tile([1, 128], FP, tag="row")
                nc.tensor.transpose(rtpT, rtp, identf)
                rs = rowp.tile([1, 1], FP, tag="rs")
                nc.vector.reduce_sum(rs, rtpT, axis=AXX)
                nc.vector.reciprocal(out=rs, in_=rs)
                hbrow = rowps.tile([1, 256], BF, tag="row")
                nc.tensor.transpose(hbrow[0:1, 0:128], hbc[:, 0:1], identb)
                nc.tensor.transpose(hbrow[0:1, 128:256], hbc[:, 1:2], identb)
                hbar_sb = rowp.tile([1, D2], BF, tag="hbar_sb")
                nc.scalar.activation(
                    out=hbar_sb[0:1, 0:128], in_=hbrow[0:1, 0:128],
                    func=COPYF, scale=rs[:, 0:1],
                )
                nc.scalar.activation(
                    out=hbar_sb[0:1, 128:D2], in_=hbrow[0:1, 184:256],
                    func=COPYF, scale=rs[:, 0:1],
                )
                psb = rowps.tile([128, D2], FP, tag="row")
                nc.tensor.matmul(psb, ones_row, hbar_sb, start=True, stop=True)
                hb_sb = rowp.tile([128, D2], FP, tag="hb_sb")
                nc.scalar.copy(out=hb_sb, in_=psb)
                st["hb_sb"] = hb_sb

            def rowfin(st, q):
                g = st["g"]
                gv = g.rearrange("p (c gg) -> p c gg", gg=DG)
                nc.gpsimd.tensor_mul(
                    gv[:, 4 * q:4 * q + 4, 3 * D2:4 * D2],
                    gv[:, 4 * q:4 * q + 4, 0:D2],
                    st["hb_sb"][:, None, :].broadcast_to([128, 4, D2]),
                )

            def new_state(r, g):
                rt_raw = rowp.tile([128, NCHUNK], BF, tag="rt_raw")
                rt = rowp.tile([128, NCHUNK], BF, tag="rt")
                return {
                    "r": r, "g": g, "rt_raw": rt_raw, "rt": rt,
                    "Hb": [None, None], "ps_s": [None, None],
                    "eT": [None, None],
                }

            # ================= cross-row pipelined schedule =================
            grows = [None] * B
            for r in range(min(4, B)):
                grows[r] = load_grow(r)
            states = [None] * B
            states[0] = new_state(0, grows[0])
            headA(states[0], 0)
            headB(states[0], 0)
            headA(states[0], 1)
            headB(states[0], 1)
            soft(states[0], 0)
            for r in range(B):
                st = states[r]
                prev = states[r - 1] if r > 0 else None
                # ---- quad 0 ----
                soft(st, 1)
                tail(st, 0)
                if r + 1 < B:
                    if states[r + 1] is None:
                        states[r + 1] = new_state(r + 1, grows[r + 1])
                    headA(states[r + 1], 0)
                    headB(states[r + 1], 0)
                if prev is not None:
                    rowfin(prev, 0)
                    store_full(r - 1, prev["g"], 0)
                if r + 2 < B and grows[r + 2] is None:
                    grows[r + 2] = load_grow(r + 2)
                # ---- quad 1 ----
                tail(st, 1)
                if r + 1 < B:
                    headA(states[r + 1], 1)
                    headB(states[r + 1], 1)
                if prev is not None:
                    rowfin(prev, 1)
                    store_full(r - 1, prev["g"], 1)
                rowend(st)
                if r + 1 < B:
                    soft(states[r + 1], 0)
            # last row epilogue: H*H_ split across GpSimd and DVE so the
            # final stores start as early as possible
            last = states[B - 1]
            g = last["g"]
            gvl = g.rearrange("p (c gg) -> p c gg", gg=DG)
            hbb = last["hb_sb"][:, None, :]
            for q in range(2):
                for h in range(2):
                    eng = nc.gpsimd if h == 0 else nc.vector
                    c2 = 4 * q + 2 * h
                    eng.tensor_mul(
                        gvl[:, c2:c2 + 2, 3 * D2:4 * D2],
                        gvl[:, c2:c2 + 2, 0:D2],
                        hbb.broadcast_to([128, 2, D2]),
                    )
                store_full(B - 1, g, q)

    if split_waits:
        _split_overwide_waits(nc)
    return nc


_NC_CACHE = None


def _get_nc():
    global _NC_CACHE
    if _NC_CACHE is None:
        _NC_CACHE = build_program()
    return _NC_CACHE


def run_sharded(inputs, trace=False):
    from concourse.bass_utils import run_bass_kernel_spmd

    H = np.ascontiguousarray(np.asarray(inputs["H"], dtype=np.float32))
    U = np.ascontiguousarray(np.asarray(inputs["U"], dtype=np.float32))
    cm = np.ascontiguousarray(np.asarray(inputs["c_mask"], dtype=np.float32))
    qm = np.ascontiguousarray(np.asarray(inputs["q_mask"], dtype=np.float32))
    w = np.ascontiguousarray(np.asarray(inputs["w"], dtype=np.float32))
    b = np.asarray(inputs["b"], dtype=np.float32).reshape(1, 1)

    nc = _get_nc()
    in_maps = []
    for c in range(N_CORES):
        s = slice(c * B, (c + 1) * B)
        in_maps.append(
            {"H": H[s], "U": U[s], "c_mask": cm[s], "q_mask": qm[s], "w": w, "b": b}
        )
    res = run_bass_kernel_spmd(
        nc, in_maps, core_ids=list(range(N_CORES)), trace=trace
    )
    G = np.concatenate([res.results[c]["G"] for c in range(N_CORES)], axis=0)
    return G, res


def kernel(H, U, c_mask, q_mask, w, b):
    G, _ = run_sharded(
        {"H": H, "U": U, "c_mask": c_mask, "q_mask": q_mask, "w": w, "b": b}
    )
    return G
